# revision 68
# baseline (speedup 1.0000x reference)
"""Causal attention head (B=8, C=2048, E=1024, H=64) with post-softmax query-row
zero mask, on 8 TRN2 NeuronCores — data-parallel over batch (one batch per core).

Sparse trick: ~50% of query rows are zero-masked post-softmax, so their outputs
are never needed. The host gathers the kept query positions (sorted), pads them
at the FRONT to a fixed QK_PAD, and the device computes attention only for
gathered query columns. Causality for gathered columns is enforced by
(a) restricting each score tile's moving range to columns whose position can
reach that key chunk (host-baked, min over cores) and (b) one narrow host-built
0/1 mask multiply per boundary tile (per-core data).

Per-core dataflow (all matmuls bf16 -> f32 PSUM):
  [Wk|Wv] packed projection over all 2048 key positions -> k_sb, vt_sb.
  v^T PE-transposed into v_aug tiles [128j, 65] with column 64 = 1.0 (the AV
        matmul then also emits softmax denominators as row 64).
  Wq projection over gathered x columns -> q_sb [64, QK_PAD].
  scoresT[j, q] = k-chunk (stationary, row-tiled across PE array halves) @ q
        (moving); exp on ScalarE with the C**-0.5 scale fused, two j-tiles per
        exp op; AV accumulates outT[65, q] over j-chunks (row 64 = softmax
        denominators). The Q path (xq, Wq) runs in fp8e4 — its quantization
        error is attenuated ~45x by the C**-0.5 score scale.
  The host divides by the denominators and scatters columns back to rows
  (masked rows stay zero) while unsharding.
"""

import numpy as np
import ml_dtypes

import concourse.bass as bass
import concourse.bacc as bacc
import concourse.mybir as mybir
import concourse.tile as tile
from concourse.bass_utils import run_bass_kernel_spmd
from concourse.masks import make_identity

B, C, E, H = 8, 2048, 1024, 64
EC = E // 128          # 8 contraction chunks
KC = C // 512          # 4 key/value column chunks of 512
NJ = C // 128          # 16 key chunks of 128
QK_PAD = 1536          # gathered queries padded (front) to this
QKC = QK_PAD // 512    # 3 gathered-query chunks
SCALE = float(C) ** -0.5
BF16 = mybir.dt.bfloat16
F32 = mybir.dt.float32

_CACHED = {}


def _plan(zero_mask):
    """Host-side plan: per-core gathered positions + shared baked bounds."""
    zm = np.asarray(zero_mask)
    pos = []   # per core: [QK_PAD] int, -1 for front pads
    for b in range(B):
        kept = np.nonzero(~zm[b])[0]
        assert len(kept) <= QK_PAD, len(kept)
        p = np.full(QK_PAD, -1, dtype=np.int64)
        p[QK_PAD - len(kept):] = kept
        pos.append(p)
    pos = np.stack(pos)  # [B, QK_PAD]

    # qoff[b, ck, jc] = #cols in chunk ck with pos < 128*jc (cols are sorted)
    qoff = np.zeros((B, QKC, NJ + 1), dtype=np.int64)
    for ck in range(QKC):
        pc = pos[:, ck * 512:(ck + 1) * 512]
        for jc in range(NJ + 1):
            qoff[:, ck, jc] = (pc < 128 * jc).sum(axis=1)
    jmax = []   # per chunk: number of key chunks any core needs
    mm_off = []  # baked matmul start col (min over cores)
    mk_end = []  # baked mask end col (max over cores)
    for ck in range(QKC):
        jm = 0
        for jc in range(NJ):
            if (qoff[:, ck, jc] < 512).any():
                jm = jc + 1
        jmax.append(jm)
        mm_off.append([int(qoff[:, ck, jc].min()) for jc in range(NJ)])
        mk_end.append([int(qoff[:, ck, jc + 1].max()) for jc in range(NJ)])
    return pos, qoff, tuple(jmax), mm_off, mk_end


def _build(jmax, mm_off, mk_end, mask_w):
    nc = bacc.Bacc("TRN2", target_bir_lowering=False, debug=False, num_devices=B)
    # host-retiled, partition-contiguous inputs (DMA descriptor gen is
    # ~4.7ns/segment on the sync sequencer, so keep segments long)
    FP8 = mybir.dt.float8e4
    xt_ext = nc.dram_tensor("xt", [128, KC * EC * 512], BF16, kind="ExternalInput")
    xq_ext = nc.dram_tensor("xq", [128, QKC * EC * 512], FP8, kind="ExternalInput")
    wkv_ext = nc.dram_tensor("wkv", [128, EC * 128], BF16, kind="ExternalInput")
    wq_ext = nc.dram_tensor("wq", [128, EC * H], FP8, kind="ExternalInput")
    msk_ext = nc.dram_tensor("msk", [128, max(mask_w, 1)], BF16, kind="ExternalInput")
    out_ext = nc.dram_tensor("out", [H + 1, QK_PAD], F32, kind="ExternalOutput")

    with tile.TileContext(nc) as tc:
        with (
            tc.tile_pool(name="const", bufs=1) as const_pool,
            tc.tile_pool(name="acts", bufs=1) as act_pool,
            tc.tile_pool(name="p", bufs=4) as p_pool,
            tc.tile_pool(name="osb", bufs=2) as o_pool,
            tc.tile_pool(name="bc", bufs=2) as bc_pool,
            tc.tile_pool(name="mmp", bufs=2, space="PSUM") as mmp_pool,
            tc.tile_pool(name="mms", bufs=2, space="PSUM") as mms_pool,
            tc.tile_pool(name="po", bufs=2, space="PSUM") as po_pool,
        ):
            wkv_sb = const_pool.tile([128, EC * 128], BF16)
            wq_sb = const_pool.tile([128, EC * H], FP8)
            msk_sb = const_pool.tile([128, max(mask_w, 1)], BF16)
            ident = const_pool.tile([128, 128], BF16)
            xt_sb = act_pool.tile([128, KC * EC * 512], BF16)
            xq_sb = act_pool.tile([128, QKC * EC * 512], FP8)

            # DMA order tuned so the kv-projection chain (the long PE pole) is
            # never starved; xq chunks arrive just before their attention use
            def dma_xt(c):
                # halves: the kv-proj e-loop can start on the first 4 e-slices
                # while the rest of the chunk streams in
                h = EC * 512 // 2
                for j in range(2):
                    nc.sync.dma_start(
                        xt_sb[:, c * EC * 512 + j * h: c * EC * 512 + (j + 1) * h],
                        xt_ext.ap()[:, c * EC * 512 + j * h: c * EC * 512 + (j + 1) * h])

            def dma_xq(c):
                nc.sync.dma_start(
                    xq_sb[:, c * EC * 512:(c + 1) * EC * 512],
                    xq_ext.ap()[:, c * EC * 512:(c + 1) * EC * 512])

            nc.sync.dma_start(wkv_sb[:], wkv_ext.ap())
            dma_xt(0)
            nc.sync.dma_start(wq_sb[:], wq_ext.ap())
            dma_xt(1)
            dma_xt(2)
            dma_xt(3)
            nc.sync.dma_start(msk_sb[:], msk_ext.ap())
            dma_xq(0)
            dma_xq(1)
            dma_xq(2)
            make_identity(nc, ident[:])
            # touch Exp once so the ACT table set loads during the DMA phase,
            # not in the middle of the attention pipeline (~2.7us)
            warm = const_pool.tile([1, 1], F32)
            nc.scalar.activation(warm[:], ident[0:1, 0:1],
                                 mybir.ActivationFunctionType.Exp)

            # k and q live duplicated in both partition halves so score matmuls
            # (K=64) can run row-tiled: even j-chunks use array rows 0:64, odd
            # j-chunks rows 64:128 — the two halves compute concurrently.
            k_sb = act_pool.tile([128, C], BF16)
            vt_sb = act_pool.tile([64, C], BF16)
            q_sb = act_pool.tile([128, QK_PAD], BF16)
            vaug_sb = act_pool.tile([128, NJ * (H + 1)], BF16)
            nc.vector.memset(vaug_sb[:], 1.0)

            def kv_proj(c):
                csl = slice(c * 512, (c + 1) * 512)
                pq = mmp_pool.tile([128, 512], F32, tag="mm")
                for e in range(EC):
                    nc.tensor.matmul(
                        pq[:], wkv_sb[:, e * 128:(e + 1) * 128],
                        xt_sb[:, (c * EC + e) * 512:(c * EC + e + 1) * 512],
                        start=(e == 0), stop=(e == EC - 1))
                nc.vector.tensor_copy(k_sb[0:64, csl], pq[0:64, :])
                nc.vector.tensor_copy(k_sb[64:128, csl], pq[0:64, :])
                nc.vector.tensor_copy(vt_sb[:, csl], pq[64:128, :])
                for jj in range(4):
                    jc = 4 * c + jj
                    pt = mmp_pool.tile([128, H], BF16, tag="mm")
                    nc.tensor.transpose(
                        pt[:], vt_sb[:, jc * 128:(jc + 1) * 128],
                        ident[0:64, 0:64])
                    nc.vector.tensor_copy(
                        vaug_sb[:, jc * (H + 1): jc * (H + 1) + H], pt[:])

            def q_proj_solo(ck):
                w0 = mm_off[ck][0]
                if w0 >= 512:
                    return
                pv = mmp_pool.tile([64, 512], F32, tag="mm")
                for e in range(EC):
                    nc.tensor.matmul(
                        pv[:, w0:512], wq_sb[:, e * H:(e + 1) * H],
                        xq_sb[:, (ck * EC + e) * 512 + w0:(ck * EC + e + 1) * 512],
                        start=(e == 0), stop=(e == EC - 1))
                nc.vector.tensor_copy(
                    q_sb[0:64, ck * 512 + w0:(ck + 1) * 512], pv[:, w0:512])
                nc.scalar.copy(
                    q_sb[64:128, ck * 512 + w0:(ck + 1) * 512], pv[:, w0:512])

            def q_proj_pair(ck_lo, ck_hi):
                # col-tiled: ck_lo accumulates in psum rows 0:64, ck_hi in
                # rows 64:128 — concurrent on the two column halves of the PE
                w1, w2 = mm_off[ck_lo][0], mm_off[ck_hi][0]
                pv = mmp_pool.tile([128, 512], F32, tag="mm")
                for e in range(EC):
                    if w1 < 512:
                        nc.tensor.matmul(
                            pv[0:64, w1:512], wq_sb[:, e * H:(e + 1) * H],
                            xq_sb[:, (ck_lo * EC + e) * 512 + w1:(ck_lo * EC + e + 1) * 512],
                            start=(e == 0), stop=(e == EC - 1),
                            skip_group_check=True)
                    if w2 < 512:
                        nc.tensor.matmul(
                            pv[64:128, w2:512], wq_sb[:, e * H:(e + 1) * H],
                            xq_sb[:, (ck_hi * EC + e) * 512 + w2:(ck_hi * EC + e + 1) * 512],
                            start=(e == 0), stop=(e == EC - 1),
                            skip_group_check=True)
                if w1 < 512:
                    sl = slice(ck_lo * 512 + w1, (ck_lo + 1) * 512)
                    nc.vector.tensor_copy(q_sb[0:64, sl], pv[0:64, w1:512])
                    nc.scalar.copy(q_sb[64:128, sl], pv[0:64, w1:512])
                if w2 < 512:
                    sl = slice(ck_hi * 512 + w2, (ck_hi + 1) * 512)
                    nc.vector.tensor_copy(q_sb[0:64, sl], pv[64:128, w2:512])
                    nc.scalar.copy(q_sb[64:128, sl], pv[64:128, w2:512])

            def attention(ck, mask_offs):
                tiles = [(jc, mm_off[ck][jc], mk_end[ck][jc])
                         for jc in range(jmax[ck]) if mm_off[ck][jc] < 512]
                if not tiles:
                    return
                po_t = po_pool.tile([H + 1, 512], F32, tag="po")
                first = True
                i = 0
                while i < len(tiles):
                    # two j-tiles share one psum + one exp, but only when the
                    # second tile is wide enough that one big exp (which also
                    # covers the [512:512+qoB] garbage span) beats two exps
                    pair = tiles[i:i + 2]
                    if len(pair) == 2 and pair[1][1] >= 352:
                        pair = tiles[i:i + 1]
                    ps = mms_pool.tile([128, 1024], F32, tag="mms")
                    p_t = p_pool.tile([128, 1024], BF16, tag="p")
                    for h, (jc, qo, me) in enumerate(pair):
                        hf = 64 * (jc % 2)  # row-tiled: alternate array halves
                        nc.tensor.matmul(
                            ps[:, h * 512 + qo:(h + 1) * 512],
                            k_sb[hf:hf + 64, jc * 128:(jc + 1) * 128],
                            q_sb[hf:hf + 64, ck * 512 + qo:(ck + 1) * 512],
                            start=True, stop=True, skip_group_check=True)
                    lo = pair[0][1]
                    hi = (len(pair) - 1) * 512 + 512
                    nc.scalar.activation(
                        p_t[:, lo:hi], ps[:, lo:hi],
                        mybir.ActivationFunctionType.Exp, scale=SCALE)
                    for h, (jc, qo, me) in enumerate(pair):
                        if me > qo:  # boundary mask multiply (host-built content)
                            mo = mask_offs[(ck, jc)]
                            nc.vector.tensor_mul(
                                p_t[:, h * 512 + qo:h * 512 + me],
                                p_t[:, h * 512 + qo:h * 512 + me],
                                msk_sb[:, mo:mo + (me - qo)])
                        nc.tensor.matmul(
                            po_t[:, qo:512],
                            vaug_sb[:, jc * (H + 1):(jc + 1) * (H + 1)],
                            p_t[:, h * 512 + qo:(h + 1) * 512],
                            start=first, stop=(i + h == len(tiles) - 1))
                        first = False
                    i += len(pair)
                # ship unnormalized outT + sums row; the host divides while
                # unsharding (removes the recip chain from the critical tail)
                w0 = mm_off[ck][0]
                o_t = o_pool.tile([H + 1, 512], F32, tag="o")
                nc.vector.tensor_copy(o_t[:, w0:512], po_t[:, w0:512])
                nc.sync.dma_start(
                    out_ext.ap()[:, ck * 512 + w0:(ck + 1) * 512], o_t[:, w0:512])

            # mask tile packing offsets (shared layout; content is per-core)
            mask_offs = {}
            off = 0
            for ck in range(QKC):
                for jc in range(jmax[ck]):
                    qo, me = mm_off[ck][jc], mk_end[ck][jc]
                    if me > qo and qo < 512:
                        mask_offs[(ck, jc)] = off
                        off += me - qo

            # schedule: kv chunks feed attention chunks as soon as possible
            need_kv = [int(np.ceil(jmax[ck] / 4)) for ck in range(QKC)]  # kv chunks needed
            done_kv = 0
            for ck in range(QKC):
                while done_kv < need_kv[ck]:
                    kv_proj(done_kv)
                    done_kv += 1
                if ck == 0:
                    q_proj_pair(0, 0)
                elif ck == 1:
                    q_proj_pair(1, 2)
                attention(ck, mask_offs)
            while done_kv < KC:
                kv_proj(done_kv)
                done_kv += 1

    nc.compile()
    return nc


def _pack_masks(pos, jmax, mm_off, mk_end):
    """Per-core packed boundary masks: msk[j_local, off+q-qo] = (128jc + j_local <= pos[q])."""
    total = 0
    spans = []
    for ck in range(len(jmax)):
        for jc in range(jmax[ck]):
            qo, me = mm_off[ck][jc], mk_end[ck][jc]
            if me > qo and qo < 512:
                spans.append((ck, jc, qo, me, total))
                total += me - qo
    bf = ml_dtypes.bfloat16
    masks = np.zeros((B, 128, max(total, 1)), dtype=np.float32)
    jl = np.arange(128)[:, None]
    for b in range(B):
        for ck, jc, qo, me, off in spans:
            pq = pos[b, ck * 512 + qo: ck * 512 + me][None, :]
            masks[b, :, off:off + (me - qo)] = (128 * jc + jl <= pq)
    return masks.astype(bf), total


def _sbufify(w):  # [E, M] -> [128, EC*M]: w_t[p, e*M+m] = w[e*128+p, m]
    M = w.shape[1]
    return np.ascontiguousarray(
        w.reshape(EC, 128, M).transpose(1, 0, 2).reshape(128, EC * M))


def _retile_cols(xt, ncols):  # [E, ncols] -> [128, (ncols/512)*EC*512] chunk-major
    return np.ascontiguousarray(
        xt.reshape(EC, 128, ncols // 512, 512).transpose(1, 2, 0, 3)
        .reshape(128, (ncols // 512) * EC * 512))


def make_in_maps(x, Wq, Wk, Wv, zero_mask):
    x = np.asarray(x)
    pos, qoff, jmax, mm_off, mk_end = _plan(zero_mask)
    masks, mask_w = _pack_masks(pos, jmax, mm_off, mk_end)
    bf = ml_dtypes.bfloat16
    f8 = ml_dtypes.float8_e4m3fn
    wkv = _sbufify(np.concatenate([np.asarray(Wk), np.asarray(Wv)], 1)).astype(bf)
    wq = _sbufify(np.asarray(Wq)).astype(f8)
    maps = []
    for b in range(B):
        xtb = np.ascontiguousarray(x[b].T.astype(np.float32))
        xqb = np.zeros((E, QK_PAD), dtype=np.float32)
        real = pos[b] >= 0
        xqb[:, real] = xtb[:, pos[b][real]]
        maps.append({
            "xt": _retile_cols(xtb, C).astype(bf),
            "xq": _retile_cols(xqb, QK_PAD).astype(f8),
            "wkv": wkv, "wq": wq, "msk": masks[b],
        })
    return maps, (pos, jmax, mm_off, mk_end, mask_w)


def kernel(x, Wq, Wk, Wv, zero_mask):
    in_maps, (pos, jmax, mm_off, mk_end, mask_w) = make_in_maps(
        x, Wq, Wk, Wv, zero_mask)
    key = (jmax, tuple(map(tuple, mm_off)), tuple(map(tuple, mk_end)), mask_w)
    if _CACHED.get("key") != key:
        _CACHED["nc"] = _build(jmax, mm_off, mk_end, mask_w)
        _CACHED["key"] = key
    res = run_bass_kernel_spmd(_CACHED["nc"], in_maps, core_ids=list(range(B)))
    out = np.zeros((B, C, H), dtype=np.float32)
    for b in range(B):
        r = res.results[b]["out"]  # [H+1, QK_PAD]; row H = softmax denominators
        real = pos[b] >= 0
        out[b][pos[b][real]] = (r[:H, real] / r[H:H + 1, real]).T
    return out


# revision 70
# speedup vs baseline: 1.0466x; 1.0466x over previous
"""Causal attention head (B=8, C=2048, E=1024, H=64) with post-softmax query-row
zero mask, on 8 TRN2 NeuronCores — data-parallel over batch (one batch per core).

Sparse trick: ~50% of query rows are zero-masked post-softmax, so their outputs
are never needed. The host gathers the kept query positions (sorted), pads them
at the FRONT to a fixed QK_PAD, and the device computes attention only for
gathered query columns. Causality for gathered columns is enforced by
(a) restricting each score tile's moving range to columns whose position can
reach that key chunk (host-baked, min over cores) and (b) one narrow host-built
0/1 mask multiply per boundary tile (per-core data).

Per-core dataflow (all matmuls bf16 -> f32 PSUM):
  [Wk|Wv] packed projection over all 2048 key positions -> k_sb, vt_sb.
  v^T PE-transposed into v_aug tiles [128j, 65] with column 64 = 1.0 (the AV
        matmul then also emits softmax denominators as row 64).
  Wq projection over gathered x columns -> q_sb [64, QK_PAD].
  scoresT[j, q] = k-chunk (stationary, row-tiled across PE array halves) @ q
        (moving); exp on ScalarE with the C**-0.5 scale fused, two j-tiles per
        exp op; AV accumulates outT[65, q] over j-chunks (row 64 = softmax
        denominators). The Q path (xq, Wq) runs in fp8e4 — its quantization
        error is attenuated ~45x by the C**-0.5 score scale.
  The host divides by the denominators and scatters columns back to rows
  (masked rows stay zero) while unsharding.
"""

import numpy as np
import ml_dtypes

import concourse.bass as bass
import concourse.bacc as bacc
import concourse.mybir as mybir
import concourse.tile as tile
from concourse.bass_utils import run_bass_kernel_spmd
from concourse.masks import make_identity

B, C, E, H = 8, 2048, 1024, 64
EC = E // 128          # 8 contraction chunks
KC = C // 512          # 4 key/value column chunks of 512
NJ = C // 128          # 16 key chunks of 128
QK_PAD = 1536          # gathered queries padded (front) to this
QKC = QK_PAD // 512    # 3 gathered-query chunks
SCALE = float(C) ** -0.5
BF16 = mybir.dt.bfloat16
F32 = mybir.dt.float32

_CACHED = {}


def _plan(zero_mask):
    """Host-side plan: per-core gathered positions + shared baked bounds."""
    zm = np.asarray(zero_mask)
    pos = []   # per core: [QK_PAD] int, -1 for front pads
    for b in range(B):
        kept = np.nonzero(~zm[b])[0]
        assert len(kept) <= QK_PAD, len(kept)
        p = np.full(QK_PAD, -1, dtype=np.int64)
        p[QK_PAD - len(kept):] = kept
        pos.append(p)
    pos = np.stack(pos)  # [B, QK_PAD]

    # qoff[b, ck, jc] = #cols in chunk ck with pos < 128*jc (cols are sorted)
    qoff = np.zeros((B, QKC, NJ + 1), dtype=np.int64)
    for ck in range(QKC):
        pc = pos[:, ck * 512:(ck + 1) * 512]
        for jc in range(NJ + 1):
            qoff[:, ck, jc] = (pc < 128 * jc).sum(axis=1)
    jmax = []   # per chunk: number of key chunks any core needs
    mm_off = []  # baked matmul start col (min over cores)
    mk_end = []  # baked mask end col (max over cores)
    for ck in range(QKC):
        jm = 0
        for jc in range(NJ):
            if (qoff[:, ck, jc] < 512).any():
                jm = jc + 1
        jmax.append(jm)
        mm_off.append([int(qoff[:, ck, jc].min()) for jc in range(NJ)])
        mk_end.append([int(qoff[:, ck, jc + 1].max()) for jc in range(NJ)])
    return pos, qoff, tuple(jmax), mm_off, mk_end


def _build(jmax, mm_off, mk_end, mask_w):
    nc = bacc.Bacc("TRN2", target_bir_lowering=False, debug=False, num_devices=B)
    # host-retiled, partition-contiguous inputs (DMA descriptor gen is
    # ~4.7ns/segment on the sync sequencer, so keep segments long)
    FP8 = mybir.dt.float8e4
    xt_ext = nc.dram_tensor("xt", [128, KC * EC * 512], BF16, kind="ExternalInput")
    xq_ext = nc.dram_tensor("xq", [128, QKC * EC * 512], FP8, kind="ExternalInput")
    wkv_ext = nc.dram_tensor("wkv", [128, EC * 128], BF16, kind="ExternalInput")
    wq_ext = nc.dram_tensor("wq", [128, EC * H], FP8, kind="ExternalInput")
    msk_ext = nc.dram_tensor("msk", [128, max(mask_w, 1)], BF16, kind="ExternalInput")
    out_ext = nc.dram_tensor("out", [H + 1, QK_PAD], F32, kind="ExternalOutput")

    with tile.TileContext(nc) as tc:
        with (
            tc.tile_pool(name="const", bufs=1) as const_pool,
            tc.tile_pool(name="acts", bufs=1) as act_pool,
            tc.tile_pool(name="p", bufs=4) as p_pool,
            tc.tile_pool(name="osb", bufs=2) as o_pool,
            tc.tile_pool(name="bc", bufs=2) as bc_pool,
            tc.tile_pool(name="mmp", bufs=2, space="PSUM") as mmp_pool,
            tc.tile_pool(name="mms", bufs=2, space="PSUM") as mms_pool,
            tc.tile_pool(name="po", bufs=2, space="PSUM") as po_pool,
        ):
            wkv_sb = const_pool.tile([128, EC * 128], BF16)
            wq_sb = const_pool.tile([128, EC * H], FP8)
            msk_sb = const_pool.tile([128, max(mask_w, 1)], BF16)
            ident = const_pool.tile([128, 128], BF16)
            xt_sb = act_pool.tile([128, KC * EC * 512], BF16)
            xq_sb = act_pool.tile([128, QKC * EC * 512], FP8)

            # DMA order tuned so the kv-projection chain (the long PE pole) is
            # never starved; xq chunks arrive just before their attention use
            def dma_xt(c):
                # halves: the kv-proj e-loop can start on the first 4 e-slices
                # while the rest of the chunk streams in
                h = EC * 512 // 2
                for j in range(2):
                    nc.sync.dma_start(
                        xt_sb[:, c * EC * 512 + j * h: c * EC * 512 + (j + 1) * h],
                        xt_ext.ap()[:, c * EC * 512 + j * h: c * EC * 512 + (j + 1) * h])

            def dma_xq(c):
                nc.sync.dma_start(
                    xq_sb[:, c * EC * 512:(c + 1) * EC * 512],
                    xq_ext.ap()[:, c * EC * 512:(c + 1) * EC * 512])

            nc.sync.dma_start(wkv_sb[:], wkv_ext.ap())
            dma_xt(0)
            nc.sync.dma_start(wq_sb[:], wq_ext.ap())
            dma_xt(1)
            dma_xq(0)
            nc.sync.dma_start(msk_sb[:], msk_ext.ap())
            dma_xt(2)
            dma_xt(3)
            dma_xq(1)
            dma_xq(2)
            make_identity(nc, ident[:])
            # touch Exp once so the ACT table set loads during the DMA phase,
            # not in the middle of the attention pipeline (~2.7us)
            warm = const_pool.tile([1, 1], F32)
            nc.scalar.activation(warm[:], ident[0:1, 0:1],
                                 mybir.ActivationFunctionType.Exp)

            # k and q live duplicated in both partition halves so score matmuls
            # (K=64) can run row-tiled: even j-chunks use array rows 0:64, odd
            # j-chunks rows 64:128 — the two halves compute concurrently.
            k_sb = act_pool.tile([128, C], BF16)
            vt_sb = act_pool.tile([64, C], BF16)
            q_sb = act_pool.tile([128, QK_PAD], BF16)
            vaug_sb = act_pool.tile([128, NJ * (H + 1)], BF16)
            nc.vector.memset(vaug_sb[:], 1.0)

            def kv_proj(c):
                csl = slice(c * 512, (c + 1) * 512)
                pq = mmp_pool.tile([128, 512], F32, tag="mm")
                for e in range(EC):
                    nc.tensor.matmul(
                        pq[:], wkv_sb[:, e * 128:(e + 1) * 128],
                        xt_sb[:, (c * EC + e) * 512:(c * EC + e + 1) * 512],
                        start=(e == 0), stop=(e == EC - 1))
                nc.vector.tensor_copy(k_sb[0:64, csl], pq[0:64, :])
                nc.vector.tensor_copy(k_sb[64:128, csl], pq[0:64, :])
                nc.vector.tensor_copy(vt_sb[:, csl], pq[64:128, :])
                for jj in range(4):
                    jc = 4 * c + jj
                    pt = mmp_pool.tile([128, H], BF16, tag="mm")
                    nc.tensor.transpose(
                        pt[:], vt_sb[:, jc * 128:(jc + 1) * 128],
                        ident[0:64, 0:64])
                    nc.vector.tensor_copy(
                        vaug_sb[:, jc * (H + 1): jc * (H + 1) + H], pt[:])

            def q_proj_solo(ck):
                w0 = mm_off[ck][0]
                if w0 >= 512:
                    return
                pv = mmp_pool.tile([64, 512], F32, tag="mm")
                for e in range(EC):
                    nc.tensor.matmul(
                        pv[:, w0:512], wq_sb[:, e * H:(e + 1) * H],
                        xq_sb[:, (ck * EC + e) * 512 + w0:(ck * EC + e + 1) * 512],
                        start=(e == 0), stop=(e == EC - 1))
                nc.vector.tensor_copy(
                    q_sb[0:64, ck * 512 + w0:(ck + 1) * 512], pv[:, w0:512])
                nc.scalar.copy(
                    q_sb[64:128, ck * 512 + w0:(ck + 1) * 512], pv[:, w0:512])

            def q_proj_pair(ck_lo, ck_hi):
                # col-tiled: ck_lo accumulates in psum rows 0:64, ck_hi in
                # rows 64:128 — concurrent on the two column halves of the PE
                w1, w2 = mm_off[ck_lo][0], mm_off[ck_hi][0]
                pv = mmp_pool.tile([128, 512], F32, tag="mm")
                for e in range(EC):
                    if w1 < 512:
                        nc.tensor.matmul(
                            pv[0:64, w1:512], wq_sb[:, e * H:(e + 1) * H],
                            xq_sb[:, (ck_lo * EC + e) * 512 + w1:(ck_lo * EC + e + 1) * 512],
                            start=(e == 0), stop=(e == EC - 1),
                            skip_group_check=True)
                    if w2 < 512:
                        nc.tensor.matmul(
                            pv[64:128, w2:512], wq_sb[:, e * H:(e + 1) * H],
                            xq_sb[:, (ck_hi * EC + e) * 512 + w2:(ck_hi * EC + e + 1) * 512],
                            start=(e == 0), stop=(e == EC - 1),
                            skip_group_check=True)
                if w1 < 512:
                    sl = slice(ck_lo * 512 + w1, (ck_lo + 1) * 512)
                    nc.vector.tensor_copy(q_sb[0:64, sl], pv[0:64, w1:512])
                    nc.scalar.copy(q_sb[64:128, sl], pv[0:64, w1:512])
                if w2 < 512:
                    sl = slice(ck_hi * 512 + w2, (ck_hi + 1) * 512)
                    nc.vector.tensor_copy(q_sb[0:64, sl], pv[64:128, w2:512])
                    nc.scalar.copy(q_sb[64:128, sl], pv[64:128, w2:512])

            def attention(ck, mask_offs):
                tiles = [(jc, mm_off[ck][jc], mk_end[ck][jc])
                         for jc in range(jmax[ck]) if mm_off[ck][jc] < 512]
                if not tiles:
                    return
                po_t = po_pool.tile([H + 1, 512], F32, tag="po")
                first = True
                i = 0
                while i < len(tiles):
                    # two j-tiles share one psum + one exp, but only when the
                    # second tile is wide enough that one big exp (which also
                    # covers the [512:512+qoB] garbage span) beats two exps
                    pair = tiles[i:i + 2]
                    if len(pair) == 2 and pair[1][1] >= 352:
                        pair = tiles[i:i + 1]
                    ps = mms_pool.tile([128, 1024], F32, tag="mms")
                    p_t = p_pool.tile([128, 1024], BF16, tag="p")
                    for h, (jc, qo, me) in enumerate(pair):
                        hf = 64 * (jc % 2)  # row-tiled: alternate array halves
                        nc.tensor.matmul(
                            ps[:, h * 512 + qo:(h + 1) * 512],
                            k_sb[hf:hf + 64, jc * 128:(jc + 1) * 128],
                            q_sb[hf:hf + 64, ck * 512 + qo:(ck + 1) * 512],
                            start=True, stop=True, skip_group_check=True)
                    lo = pair[0][1]
                    hi = (len(pair) - 1) * 512 + 512
                    nc.scalar.activation(
                        p_t[:, lo:hi], ps[:, lo:hi],
                        mybir.ActivationFunctionType.Exp, scale=SCALE)
                    for h, (jc, qo, me) in enumerate(pair):
                        if me > qo:  # boundary mask multiply (host-built content)
                            mo = mask_offs[(ck, jc)]
                            nc.vector.tensor_mul(
                                p_t[:, h * 512 + qo:h * 512 + me],
                                p_t[:, h * 512 + qo:h * 512 + me],
                                msk_sb[:, mo:mo + (me - qo)])
                        nc.tensor.matmul(
                            po_t[:, qo:512],
                            vaug_sb[:, jc * (H + 1):(jc + 1) * (H + 1)],
                            p_t[:, h * 512 + qo:(h + 1) * 512],
                            start=first, stop=(i + h == len(tiles) - 1))
                        first = False
                    i += len(pair)
                # ship unnormalized outT + sums row; the host divides while
                # unsharding (removes the recip chain from the critical tail)
                w0 = mm_off[ck][0]
                o_t = o_pool.tile([H + 1, 512], F32, tag="o")
                nc.vector.tensor_copy(o_t[:, w0:512], po_t[:, w0:512])
                nc.sync.dma_start(
                    out_ext.ap()[:, ck * 512 + w0:(ck + 1) * 512], o_t[:, w0:512])

            # mask tile packing offsets (shared layout; content is per-core)
            mask_offs = {}
            off = 0
            for ck in range(QKC):
                for jc in range(jmax[ck]):
                    qo, me = mm_off[ck][jc], mk_end[ck][jc]
                    if me > qo and qo < 512:
                        mask_offs[(ck, jc)] = off
                        off += me - qo

            # schedule: kv chunks feed attention chunks as soon as possible
            need_kv = [int(np.ceil(jmax[ck] / 4)) for ck in range(QKC)]  # kv chunks needed
            done_kv = 0
            for ck in range(QKC):
                while done_kv < need_kv[ck]:
                    kv_proj(done_kv)
                    done_kv += 1
                if ck == 0:
                    q_proj_pair(0, 0)
                else:
                    q_proj_solo(ck)
                attention(ck, mask_offs)
            while done_kv < KC:
                kv_proj(done_kv)
                done_kv += 1

    nc.compile()
    return nc


def _pack_masks(pos, jmax, mm_off, mk_end):
    """Per-core packed boundary masks: msk[j_local, off+q-qo] = (128jc + j_local <= pos[q])."""
    total = 0
    spans = []
    for ck in range(len(jmax)):
        for jc in range(jmax[ck]):
            qo, me = mm_off[ck][jc], mk_end[ck][jc]
            if me > qo and qo < 512:
                spans.append((ck, jc, qo, me, total))
                total += me - qo
    bf = ml_dtypes.bfloat16
    masks = np.zeros((B, 128, max(total, 1)), dtype=np.float32)
    jl = np.arange(128)[:, None]
    for b in range(B):
        for ck, jc, qo, me, off in spans:
            pq = pos[b, ck * 512 + qo: ck * 512 + me][None, :]
            masks[b, :, off:off + (me - qo)] = (128 * jc + jl <= pq)
    return masks.astype(bf), total


def _sbufify(w):  # [E, M] -> [128, EC*M]: w_t[p, e*M+m] = w[e*128+p, m]
    M = w.shape[1]
    return np.ascontiguousarray(
        w.reshape(EC, 128, M).transpose(1, 0, 2).reshape(128, EC * M))


def _retile_cols(xt, ncols):  # [E, ncols] -> [128, (ncols/512)*EC*512] chunk-major
    return np.ascontiguousarray(
        xt.reshape(EC, 128, ncols // 512, 512).transpose(1, 2, 0, 3)
        .reshape(128, (ncols // 512) * EC * 512))


def make_in_maps(x, Wq, Wk, Wv, zero_mask):
    x = np.asarray(x)
    pos, qoff, jmax, mm_off, mk_end = _plan(zero_mask)
    masks, mask_w = _pack_masks(pos, jmax, mm_off, mk_end)
    bf = ml_dtypes.bfloat16
    f8 = ml_dtypes.float8_e4m3fn
    wkv = _sbufify(np.concatenate([np.asarray(Wk), np.asarray(Wv)], 1)).astype(bf)
    wq = _sbufify(np.asarray(Wq)).astype(f8)
    maps = []
    for b in range(B):
        xtb = np.ascontiguousarray(x[b].T.astype(np.float32))
        xqb = np.zeros((E, QK_PAD), dtype=np.float32)
        real = pos[b] >= 0
        xqb[:, real] = xtb[:, pos[b][real]]
        maps.append({
            "xt": _retile_cols(xtb, C).astype(bf),
            "xq": _retile_cols(xqb, QK_PAD).astype(f8),
            "wkv": wkv, "wq": wq, "msk": masks[b],
        })
    return maps, (pos, jmax, mm_off, mk_end, mask_w)


def kernel(x, Wq, Wk, Wv, zero_mask):
    in_maps, (pos, jmax, mm_off, mk_end, mask_w) = make_in_maps(
        x, Wq, Wk, Wv, zero_mask)
    key = (jmax, tuple(map(tuple, mm_off)), tuple(map(tuple, mk_end)), mask_w)
    if _CACHED.get("key") != key:
        _CACHED["nc"] = _build(jmax, mm_off, mk_end, mask_w)
        _CACHED["key"] = key
    res = run_bass_kernel_spmd(_CACHED["nc"], in_maps, core_ids=list(range(B)))
    out = np.zeros((B, C, H), dtype=np.float32)
    for b in range(B):
        r = res.results[b]["out"]  # [H+1, QK_PAD]; row H = softmax denominators
        real = pos[b] >= 0
        out[b][pos[b][real]] = (r[:H, real] / r[H:H + 1, real]).T
    return out


# revision 71
# speedup vs baseline: 1.1077x; 1.0584x over previous
"""Causal attention head (B=8, C=2048, E=1024, H=64) with post-softmax query-row
zero mask, on 8 TRN2 NeuronCores — data-parallel over batch (one batch per core).

Sparse trick: ~50% of query rows are zero-masked post-softmax, so their outputs
are never needed. The host gathers the kept query positions (sorted), pads them
at the FRONT to a fixed QK_PAD, and the device computes attention only for
gathered query columns. Causality for gathered columns is enforced by
(a) restricting each score tile's moving range to columns whose position can
reach that key chunk (host-baked, min over cores) and (b) one narrow host-built
0/1 mask multiply per boundary tile (per-core data).

Per-core dataflow (all matmuls bf16 -> f32 PSUM):
  [Wk|Wv] packed projection over all 2048 key positions -> k_sb, vt_sb.
  v^T PE-transposed into v_aug tiles [128j, 65] with column 64 = 1.0 (the AV
        matmul then also emits softmax denominators as row 64).
  Wq projection over gathered x columns -> q_sb [64, QK_PAD].
  scoresT[j, q] = k-chunk (stationary, row-tiled across PE array halves) @ q
        (moving); exp on ScalarE with the C**-0.5 scale fused, two j-tiles per
        exp op; AV accumulates outT[65, q] over j-chunks (row 64 = softmax
        denominators). The Q path (xq, Wq) runs in fp8e4 — its quantization
        error is attenuated ~45x by the C**-0.5 score scale.
  The host divides by the denominators and scatters columns back to rows
  (masked rows stay zero) while unsharding.
"""

import numpy as np
import ml_dtypes

import concourse.bass as bass
import concourse.bacc as bacc
import concourse.mybir as mybir
import concourse.tile as tile
from concourse.bass_utils import run_bass_kernel_spmd
from concourse.masks import make_identity

B, C, E, H = 8, 2048, 1024, 64
EC = E // 128          # 8 contraction chunks
KC = C // 512          # 4 key/value column chunks of 512
NJ = C // 128          # 16 key chunks of 128
QK_PAD = 1536          # gathered queries padded (front) to this
QKC = QK_PAD // 512    # 3 gathered-query chunks
SCALE = float(C) ** -0.5
BF16 = mybir.dt.bfloat16
F32 = mybir.dt.float32

_CACHED = {}


def _plan(zero_mask):
    """Host-side plan: per-core gathered positions + shared baked bounds."""
    zm = np.asarray(zero_mask)
    pos = []   # per core: [QK_PAD] int, -1 for front pads
    for b in range(B):
        kept = np.nonzero(~zm[b])[0]
        assert len(kept) <= QK_PAD, len(kept)
        p = np.full(QK_PAD, -1, dtype=np.int64)
        p[QK_PAD - len(kept):] = kept
        pos.append(p)
    pos = np.stack(pos)  # [B, QK_PAD]

    # qoff[b, ck, jc] = #cols in chunk ck with pos < 128*jc (cols are sorted)
    qoff = np.zeros((B, QKC, NJ + 1), dtype=np.int64)
    for ck in range(QKC):
        pc = pos[:, ck * 512:(ck + 1) * 512]
        for jc in range(NJ + 1):
            qoff[:, ck, jc] = (pc < 128 * jc).sum(axis=1)
    jmax = []   # per chunk: number of key chunks any core needs
    mm_off = []  # baked matmul start col (min over cores)
    mk_end = []  # baked mask end col (max over cores)
    for ck in range(QKC):
        jm = 0
        for jc in range(NJ):
            if (qoff[:, ck, jc] < 512).any():
                jm = jc + 1
        jmax.append(jm)
        mm_off.append([int(qoff[:, ck, jc].min()) for jc in range(NJ)])
        mk_end.append([int(qoff[:, ck, jc + 1].max()) for jc in range(NJ)])
    return pos, qoff, tuple(jmax), mm_off, mk_end


def _build(jmax, mm_off, mk_end, mask_w):
    nc = bacc.Bacc("TRN2", target_bir_lowering=False, debug=False, num_devices=B)
    # host-retiled, partition-contiguous inputs (DMA descriptor gen is
    # ~4.7ns/segment on the sync sequencer, so keep segments long)
    FP8 = mybir.dt.float8e4
    xt_ext = nc.dram_tensor("xt", [128, KC * EC * 512], BF16, kind="ExternalInput")
    xq_ext = nc.dram_tensor("xq", [128, QKC * EC * 512], FP8, kind="ExternalInput")
    wkv_ext = nc.dram_tensor("wkv", [128, EC * 128], BF16, kind="ExternalInput")
    wq_ext = nc.dram_tensor("wq", [128, EC * H], FP8, kind="ExternalInput")
    msk_ext = nc.dram_tensor("msk", [128, max(mask_w, 1)], BF16, kind="ExternalInput")
    out_ext = nc.dram_tensor("out", [H + 1, QK_PAD], F32, kind="ExternalOutput")

    with tile.TileContext(nc) as tc:
        with (
            tc.tile_pool(name="const", bufs=1) as const_pool,
            tc.tile_pool(name="acts", bufs=1) as act_pool,
            tc.tile_pool(name="p", bufs=4) as p_pool,
            tc.tile_pool(name="osb", bufs=2) as o_pool,
            tc.tile_pool(name="bc", bufs=2) as bc_pool,
            tc.tile_pool(name="mmp", bufs=2, space="PSUM") as mmp_pool,
            tc.tile_pool(name="mms", bufs=2, space="PSUM") as mms_pool,
            tc.tile_pool(name="po", bufs=2, space="PSUM") as po_pool,
        ):
            wkv_sb = const_pool.tile([128, EC * 128], BF16)
            wq_sb = const_pool.tile([128, EC * H], FP8)
            msk_sb = const_pool.tile([128, max(mask_w, 1)], BF16)
            ident = const_pool.tile([128, 128], BF16)
            xt_sb = act_pool.tile([128, KC * EC * 512], BF16)
            xq_sb = act_pool.tile([128, QKC * EC * 512], FP8)

            # DMA order tuned so the kv-projection chain (the long PE pole) is
            # never starved; xq chunks arrive just before their attention use
            def dma_xt(c):
                # halves: the kv-proj e-loop can start on the first 4 e-slices
                # while the rest of the chunk streams in
                h = EC * 512 // 2
                for j in range(2):
                    nc.sync.dma_start(
                        xt_sb[:, c * EC * 512 + j * h: c * EC * 512 + (j + 1) * h],
                        xt_ext.ap()[:, c * EC * 512 + j * h: c * EC * 512 + (j + 1) * h])

            def dma_xq(c):
                nc.sync.dma_start(
                    xq_sb[:, c * EC * 512:(c + 1) * EC * 512],
                    xq_ext.ap()[:, c * EC * 512:(c + 1) * EC * 512])

            nc.sync.dma_start(wkv_sb[:], wkv_ext.ap())
            dma_xt(0)
            nc.sync.dma_start(wq_sb[:], wq_ext.ap())
            dma_xt(1)
            dma_xq(0)
            nc.sync.dma_start(msk_sb[:], msk_ext.ap())
            dma_xt(2)
            dma_xt(3)
            dma_xq(1)
            dma_xq(2)
            make_identity(nc, ident[:])
            # touch Exp once so the ACT table set loads during the DMA phase,
            # not in the middle of the attention pipeline (~2.7us)
            warm = const_pool.tile([1, 1], F32)
            nc.scalar.activation(warm[:], ident[0:1, 0:1],
                                 mybir.ActivationFunctionType.Exp)

            # k and q live duplicated in both partition halves so score matmuls
            # (K=64) can run row-tiled: even j-chunks use array rows 0:64, odd
            # j-chunks rows 64:128 — the two halves compute concurrently.
            k_sb = act_pool.tile([128, C], BF16)
            vt_sb = act_pool.tile([64, C], BF16)
            q_sb = act_pool.tile([128, QK_PAD], BF16)
            vaug_sb = act_pool.tile([128, NJ * (H + 1)], BF16)
            nc.vector.memset(vaug_sb[:], 1.0)

            def kv_proj(c):
                csl = slice(c * 512, (c + 1) * 512)
                pq = mmp_pool.tile([128, 512], F32, tag="mm")
                for e in range(EC):
                    nc.tensor.matmul(
                        pq[:], wkv_sb[:, e * 128:(e + 1) * 128],
                        xt_sb[:, (c * EC + e) * 512:(c * EC + e + 1) * 512],
                        start=(e == 0), stop=(e == EC - 1))
                nc.vector.tensor_copy(k_sb[0:64, csl], pq[0:64, :])
                nc.vector.tensor_copy(k_sb[64:128, csl], pq[0:64, :])
                nc.vector.tensor_copy(vt_sb[:, csl], pq[64:128, :])
                for jj in range(4):
                    jc = 4 * c + jj
                    pt = mmp_pool.tile([128, H], BF16, tag="mm")
                    nc.tensor.transpose(
                        pt[:], vt_sb[:, jc * 128:(jc + 1) * 128],
                        ident[0:64, 0:64])
                    nc.vector.tensor_copy(
                        vaug_sb[:, jc * (H + 1): jc * (H + 1) + H], pt[:])

            def q_proj_solo(ck):
                w0 = mm_off[ck][0]
                if w0 >= 512:
                    return
                pv = mmp_pool.tile([64, 512], F32, tag="mm")
                for e in range(EC):
                    nc.tensor.matmul(
                        pv[:, w0:512], wq_sb[:, e * H:(e + 1) * H],
                        xq_sb[:, (ck * EC + e) * 512 + w0:(ck * EC + e + 1) * 512],
                        start=(e == 0), stop=(e == EC - 1))
                nc.vector.tensor_copy(
                    q_sb[0:64, ck * 512 + w0:(ck + 1) * 512], pv[:, w0:512])
                nc.scalar.copy(
                    q_sb[64:128, ck * 512 + w0:(ck + 1) * 512], pv[:, w0:512])

            def q_proj_pair(ck_lo, ck_hi):
                # col-tiled: ck_lo accumulates in psum rows 0:64, ck_hi in
                # rows 64:128 — concurrent on the two column halves of the PE
                w1, w2 = mm_off[ck_lo][0], mm_off[ck_hi][0]
                pv = mmp_pool.tile([128, 512], F32, tag="mm")
                for e in range(EC):
                    if w1 < 512:
                        nc.tensor.matmul(
                            pv[0:64, w1:512], wq_sb[:, e * H:(e + 1) * H],
                            xq_sb[:, (ck_lo * EC + e) * 512 + w1:(ck_lo * EC + e + 1) * 512],
                            start=(e == 0), stop=(e == EC - 1),
                            skip_group_check=True)
                    if w2 < 512:
                        nc.tensor.matmul(
                            pv[64:128, w2:512], wq_sb[:, e * H:(e + 1) * H],
                            xq_sb[:, (ck_hi * EC + e) * 512 + w2:(ck_hi * EC + e + 1) * 512],
                            start=(e == 0), stop=(e == EC - 1),
                            skip_group_check=True)
                if w1 < 512:
                    sl = slice(ck_lo * 512 + w1, (ck_lo + 1) * 512)
                    nc.vector.tensor_copy(q_sb[0:64, sl], pv[0:64, w1:512])
                    nc.scalar.copy(q_sb[64:128, sl], pv[0:64, w1:512])
                if w2 < 512:
                    sl = slice(ck_hi * 512 + w2, (ck_hi + 1) * 512)
                    nc.vector.tensor_copy(q_sb[0:64, sl], pv[64:128, w2:512])
                    nc.scalar.copy(q_sb[64:128, sl], pv[64:128, w2:512])

            def attention(ck, mask_offs):
                tiles = [(jc, mm_off[ck][jc], mk_end[ck][jc])
                         for jc in range(jmax[ck]) if mm_off[ck][jc] < 512]
                if not tiles:
                    return
                po_t = po_pool.tile([H + 1, 512], F32, tag="po")
                first = True
                i = 0
                while i < len(tiles):
                    # two j-tiles share one psum + one exp, but only when the
                    # second tile is wide enough that one big exp (which also
                    # covers the [512:512+qoB] garbage span) beats two exps
                    pair = tiles[i:i + 2]
                    if len(pair) == 2 and pair[1][1] >= 352:
                        pair = tiles[i:i + 1]
                    ps = mms_pool.tile([128, 1024], F32, tag="mms")
                    p_t = p_pool.tile([128, 1024], BF16, tag="p")
                    for h, (jc, qo, me) in enumerate(pair):
                        hf = 64 * (jc % 2)  # row-tiled: alternate array halves
                        nc.tensor.matmul(
                            ps[:, h * 512 + qo:(h + 1) * 512],
                            k_sb[hf:hf + 64, jc * 128:(jc + 1) * 128],
                            q_sb[hf:hf + 64, ck * 512 + qo:(ck + 1) * 512],
                            start=True, stop=True, skip_group_check=True)
                    lo = pair[0][1]
                    hi = (len(pair) - 1) * 512 + 512
                    nc.scalar.activation(
                        p_t[:, lo:hi], ps[:, lo:hi],
                        mybir.ActivationFunctionType.Exp, scale=SCALE)
                    for h, (jc, qo, me) in enumerate(pair):
                        if me > qo:  # boundary mask multiply (host-built content)
                            mo = mask_offs[(ck, jc)]
                            nc.vector.tensor_mul(
                                p_t[:, h * 512 + qo:h * 512 + me],
                                p_t[:, h * 512 + qo:h * 512 + me],
                                msk_sb[:, mo:mo + (me - qo)])
                        nc.tensor.matmul(
                            po_t[:, qo:512],
                            vaug_sb[:, jc * (H + 1):(jc + 1) * (H + 1)],
                            p_t[:, h * 512 + qo:(h + 1) * 512],
                            start=first, stop=(i + h == len(tiles) - 1))
                        first = False
                    i += len(pair)
                # ship unnormalized outT + sums row; the host divides while
                # unsharding (removes the recip chain from the critical tail)
                w0 = mm_off[ck][0]
                o_t = o_pool.tile([H + 1, 512], F32, tag="o")
                nc.vector.tensor_copy(o_t[:, w0:512], po_t[:, w0:512])
                nc.sync.dma_start(
                    out_ext.ap()[:, ck * 512 + w0:(ck + 1) * 512], o_t[:, w0:512])

            # mask tile packing offsets (shared layout; content is per-core)
            mask_offs = {}
            off = 0
            for ck in range(QKC):
                for jc in range(jmax[ck]):
                    qo, me = mm_off[ck][jc], mk_end[ck][jc]
                    if me > qo and qo < 512:
                        mask_offs[(ck, jc)] = off
                        off += me - qo

            # schedule: kv chunks feed attention chunks as soon as possible
            need_kv = [int(np.ceil(jmax[ck] / 4)) for ck in range(QKC)]  # kv chunks needed
            done_kv = 0
            for ck in range(QKC):
                while done_kv < need_kv[ck]:
                    kv_proj(done_kv)
                    done_kv += 1
                if ck == 0:
                    q_proj_pair(0, 0)
                elif ck == 1:
                    q_proj_pair(1, 2)
                attention(ck, mask_offs)
            while done_kv < KC:
                kv_proj(done_kv)
                done_kv += 1

    nc.compile()
    return nc


def _pack_masks(pos, jmax, mm_off, mk_end):
    """Per-core packed boundary masks: msk[j_local, off+q-qo] = (128jc + j_local <= pos[q])."""
    total = 0
    spans = []
    for ck in range(len(jmax)):
        for jc in range(jmax[ck]):
            qo, me = mm_off[ck][jc], mk_end[ck][jc]
            if me > qo and qo < 512:
                spans.append((ck, jc, qo, me, total))
                total += me - qo
    bf = ml_dtypes.bfloat16
    masks = np.zeros((B, 128, max(total, 1)), dtype=np.float32)
    jl = np.arange(128)[:, None]
    for b in range(B):
        for ck, jc, qo, me, off in spans:
            pq = pos[b, ck * 512 + qo: ck * 512 + me][None, :]
            masks[b, :, off:off + (me - qo)] = (128 * jc + jl <= pq)
    return masks.astype(bf), total


def _sbufify(w):  # [E, M] -> [128, EC*M]: w_t[p, e*M+m] = w[e*128+p, m]
    M = w.shape[1]
    return np.ascontiguousarray(
        w.reshape(EC, 128, M).transpose(1, 0, 2).reshape(128, EC * M))


def _retile_cols(xt, ncols):  # [E, ncols] -> [128, (ncols/512)*EC*512] chunk-major
    return np.ascontiguousarray(
        xt.reshape(EC, 128, ncols // 512, 512).transpose(1, 2, 0, 3)
        .reshape(128, (ncols // 512) * EC * 512))


def make_in_maps(x, Wq, Wk, Wv, zero_mask):
    x = np.asarray(x)
    pos, qoff, jmax, mm_off, mk_end = _plan(zero_mask)
    masks, mask_w = _pack_masks(pos, jmax, mm_off, mk_end)
    bf = ml_dtypes.bfloat16
    f8 = ml_dtypes.float8_e4m3fn
    wkv = _sbufify(np.concatenate([np.asarray(Wk), np.asarray(Wv)], 1)).astype(bf)
    wq = _sbufify(np.asarray(Wq)).astype(f8)
    maps = []
    for b in range(B):
        xtb = np.ascontiguousarray(x[b].T.astype(np.float32))
        xqb = np.zeros((E, QK_PAD), dtype=np.float32)
        real = pos[b] >= 0
        xqb[:, real] = xtb[:, pos[b][real]]
        maps.append({
            "xt": _retile_cols(xtb, C).astype(bf),
            "xq": _retile_cols(xqb, QK_PAD).astype(f8),
            "wkv": wkv, "wq": wq, "msk": masks[b],
        })
    return maps, (pos, jmax, mm_off, mk_end, mask_w)


def kernel(x, Wq, Wk, Wv, zero_mask):
    in_maps, (pos, jmax, mm_off, mk_end, mask_w) = make_in_maps(
        x, Wq, Wk, Wv, zero_mask)
    key = (jmax, tuple(map(tuple, mm_off)), tuple(map(tuple, mk_end)), mask_w)
    if _CACHED.get("key") != key:
        _CACHED["nc"] = _build(jmax, mm_off, mk_end, mask_w)
        _CACHED["key"] = key
    res = run_bass_kernel_spmd(_CACHED["nc"], in_maps, core_ids=list(range(B)))
    out = np.zeros((B, C, H), dtype=np.float32)
    for b in range(B):
        r = res.results[b]["out"]  # [H+1, QK_PAD]; row H = softmax denominators
        real = pos[b] >= 0
        out[b][pos[b][real]] = (r[:H, real] / r[H:H + 1, real]).T
    return out


# revision 72
# speedup vs baseline: 1.1306x; 1.0207x over previous
"""Causal attention head (B=8, C=2048, E=1024, H=64) with post-softmax query-row
zero mask, on 8 TRN2 NeuronCores — data-parallel over batch (one batch per core).

Sparse trick: ~50% of query rows are zero-masked post-softmax, so their outputs
are never needed. The host gathers the kept query positions (sorted), pads them
at the FRONT to a fixed QK_PAD, and the device computes attention only for
gathered query columns. Causality for gathered columns is enforced by
(a) restricting each score tile's moving range to columns whose position can
reach that key chunk (host-baked, min over cores) and (b) one narrow host-built
0/1 mask multiply per boundary tile (per-core data).

Per-core dataflow (all matmuls bf16 -> f32 PSUM):
  [Wk|Wv] packed projection over all 2048 key positions -> k_sb, vt_sb.
  v^T PE-transposed into v_aug tiles [128j, 65] with column 64 = 1.0 (the AV
        matmul then also emits softmax denominators as row 64).
  Wq projection over gathered x columns -> q_sb [64, QK_PAD].
  scoresT[j, q] = k-chunk (stationary, row-tiled across PE array halves) @ q
        (moving); exp on ScalarE with the C**-0.5 scale fused, two j-tiles per
        exp op; AV accumulates outT[65, q] over j-chunks (row 64 = softmax
        denominators). The Q path (xq, Wq) runs in fp8e4 — its quantization
        error is attenuated ~45x by the C**-0.5 score scale.
  The host divides by the denominators and scatters columns back to rows
  (masked rows stay zero) while unsharding.
"""

import numpy as np
import ml_dtypes

import concourse.bass as bass
import concourse.bacc as bacc
import concourse.mybir as mybir
import concourse.tile as tile
from concourse.bass_utils import run_bass_kernel_spmd
from concourse.masks import make_identity

B, C, E, H = 8, 2048, 1024, 64
EC = E // 128          # 8 contraction chunks
KC = C // 512          # 4 key/value column chunks of 512
NJ = C // 128          # 16 key chunks of 128
QK_PAD = 1536          # gathered queries padded (front) to this
QKC = QK_PAD // 512    # 3 gathered-query chunks
SCALE = float(C) ** -0.5
BF16 = mybir.dt.bfloat16
F32 = mybir.dt.float32

_CACHED = {}


def _plan(zero_mask):
    """Host-side plan: per-core gathered positions + shared baked bounds."""
    zm = np.asarray(zero_mask)
    pos = []   # per core: [QK_PAD] int, -1 for front pads
    for b in range(B):
        kept = np.nonzero(~zm[b])[0]
        assert len(kept) <= QK_PAD, len(kept)
        p = np.full(QK_PAD, -1, dtype=np.int64)
        p[QK_PAD - len(kept):] = kept
        pos.append(p)
    pos = np.stack(pos)  # [B, QK_PAD]

    # qoff[b, ck, jc] = #cols in chunk ck with pos < 128*jc (cols are sorted)
    qoff = np.zeros((B, QKC, NJ + 1), dtype=np.int64)
    for ck in range(QKC):
        pc = pos[:, ck * 512:(ck + 1) * 512]
        for jc in range(NJ + 1):
            qoff[:, ck, jc] = (pc < 128 * jc).sum(axis=1)
    jmax = []   # per chunk: number of key chunks any core needs
    mm_off = []  # baked matmul start col (min over cores)
    mk_end = []  # baked mask end col (max over cores)
    for ck in range(QKC):
        jm = 0
        for jc in range(NJ):
            if (qoff[:, ck, jc] < 512).any():
                jm = jc + 1
        jmax.append(jm)
        mm_off.append([int(qoff[:, ck, jc].min()) for jc in range(NJ)])
        mk_end.append([int(qoff[:, ck, jc + 1].max()) for jc in range(NJ)])
    return pos, qoff, tuple(jmax), mm_off, mk_end


def _build(jmax, mm_off, mk_end, mask_w):
    nc = bacc.Bacc("TRN2", target_bir_lowering=False, debug=False, num_devices=B)
    # host-retiled, partition-contiguous inputs (DMA descriptor gen is
    # ~4.7ns/segment on the sync sequencer, so keep segments long)
    FP8 = mybir.dt.float8e4
    xt_ext = nc.dram_tensor("xt", [128, KC * EC * 512], BF16, kind="ExternalInput")
    xq_ext = nc.dram_tensor("xq", [128, QKC * EC * 512], FP8, kind="ExternalInput")
    wkv_ext = nc.dram_tensor("wkv", [128, EC * 128], BF16, kind="ExternalInput")
    wq_ext = nc.dram_tensor("wq", [128, EC * H], FP8, kind="ExternalInput")
    msk_ext = nc.dram_tensor("msk", [128, max(mask_w, 1)], BF16, kind="ExternalInput")
    out_ext = nc.dram_tensor("out", [H + 1, QK_PAD], F32, kind="ExternalOutput")

    with tile.TileContext(nc) as tc:
        with (
            tc.tile_pool(name="const", bufs=1) as const_pool,
            tc.tile_pool(name="acts", bufs=1) as act_pool,
            tc.tile_pool(name="p", bufs=4) as p_pool,
            tc.tile_pool(name="osb", bufs=2) as o_pool,
            tc.tile_pool(name="bc", bufs=2) as bc_pool,
            tc.tile_pool(name="mmp", bufs=2, space="PSUM") as mmp_pool,
            tc.tile_pool(name="mms", bufs=2, space="PSUM") as mms_pool,
            tc.tile_pool(name="po", bufs=2, space="PSUM") as po_pool,
        ):
            wkv_sb = const_pool.tile([128, EC * 128], BF16)
            wq_sb = const_pool.tile([128, EC * H], FP8)
            msk_sb = const_pool.tile([128, max(mask_w, 1)], BF16)
            ident = const_pool.tile([128, 128], BF16)
            xt_sb = act_pool.tile([128, KC * EC * 512], BF16)
            xq_sb = act_pool.tile([128, QKC * EC * 512], FP8)

            # DMA order tuned so the kv-projection chain (the long PE pole) is
            # never starved; xq chunks arrive just before their attention use
            def dma_xt(c):
                # halves: the kv-proj e-loop can start on the first 4 e-slices
                # while the rest of the chunk streams in
                h = EC * 512 // 2
                for j in range(2):
                    nc.sync.dma_start(
                        xt_sb[:, c * EC * 512 + j * h: c * EC * 512 + (j + 1) * h],
                        xt_ext.ap()[:, c * EC * 512 + j * h: c * EC * 512 + (j + 1) * h])

            def dma_xq(c):
                nc.sync.dma_start(
                    xq_sb[:, c * EC * 512:(c + 1) * EC * 512],
                    xq_ext.ap()[:, c * EC * 512:(c + 1) * EC * 512])

            nc.sync.dma_start(wkv_sb[:], wkv_ext.ap())
            dma_xt(0)
            nc.sync.dma_start(wq_sb[:], wq_ext.ap())
            dma_xt(1)
            dma_xq(0)
            nc.sync.dma_start(msk_sb[:], msk_ext.ap())
            dma_xt(2)
            dma_xt(3)
            dma_xq(1)
            dma_xq(2)
            make_identity(nc, ident[:])
            # touch Exp once so the ACT table set loads during the DMA phase,
            # not in the middle of the attention pipeline (~2.7us)
            warm = const_pool.tile([1, 1], F32)
            nc.scalar.activation(warm[:], ident[0:1, 0:1],
                                 mybir.ActivationFunctionType.Exp)

            # k and q live duplicated in both partition halves so score matmuls
            # (K=64) can run row-tiled: even j-chunks use array rows 0:64, odd
            # j-chunks rows 64:128 — the two halves compute concurrently.
            k_sb = act_pool.tile([128, C], BF16)
            vt_sb = act_pool.tile([64, C], BF16)
            q_sb = act_pool.tile([128, QK_PAD], BF16)
            vaug_sb = act_pool.tile([128, NJ * (H + 1)], BF16)
            nc.vector.memset(vaug_sb[:], 1.0)

            def kv_proj(c):
                csl = slice(c * 512, (c + 1) * 512)
                pq = mmp_pool.tile([128, 512], F32, tag="mm")
                for e in range(EC):
                    nc.tensor.matmul(
                        pq[:], wkv_sb[:, e * 128:(e + 1) * 128],
                        xt_sb[:, (c * EC + e) * 512:(c * EC + e + 1) * 512],
                        start=(e == 0), stop=(e == EC - 1))
                nc.vector.tensor_copy(k_sb[0:64, csl], pq[0:64, :])
                nc.vector.tensor_copy(k_sb[64:128, csl], pq[0:64, :])
                nc.vector.tensor_copy(vt_sb[:, csl], pq[64:128, :])
                for jj in range(4):
                    jc = 4 * c + jj
                    pt = mmp_pool.tile([128, H], BF16, tag="mm")
                    nc.tensor.transpose(
                        pt[:], vt_sb[:, jc * 128:(jc + 1) * 128],
                        ident[0:64, 0:64])
                    nc.vector.tensor_copy(
                        vaug_sb[:, jc * (H + 1): jc * (H + 1) + H], pt[:])

            def q_proj_solo(ck):
                w0 = mm_off[ck][0]
                if w0 >= 512:
                    return
                pv = mmp_pool.tile([64, 512], F32, tag="mm")
                for e in range(EC):
                    nc.tensor.matmul(
                        pv[:, w0:512], wq_sb[:, e * H:(e + 1) * H],
                        xq_sb[:, (ck * EC + e) * 512 + w0:(ck * EC + e + 1) * 512],
                        start=(e == 0), stop=(e == EC - 1))
                nc.vector.tensor_copy(
                    q_sb[0:64, ck * 512 + w0:(ck + 1) * 512], pv[:, w0:512])
                nc.scalar.copy(
                    q_sb[64:128, ck * 512 + w0:(ck + 1) * 512], pv[:, w0:512])

            def q_proj_pair(ck_lo, ck_hi):
                # col-tiled: ck_lo accumulates in psum rows 0:64, ck_hi in
                # rows 64:128 — concurrent on the two column halves of the PE
                w1, w2 = mm_off[ck_lo][0], mm_off[ck_hi][0]
                pv = mmp_pool.tile([128, 512], F32, tag="mm")
                for e in range(EC):
                    if w1 < 512:
                        nc.tensor.matmul(
                            pv[0:64, w1:512], wq_sb[:, e * H:(e + 1) * H],
                            xq_sb[:, (ck_lo * EC + e) * 512 + w1:(ck_lo * EC + e + 1) * 512],
                            start=(e == 0), stop=(e == EC - 1),
                            skip_group_check=True)
                    if w2 < 512:
                        nc.tensor.matmul(
                            pv[64:128, w2:512], wq_sb[:, e * H:(e + 1) * H],
                            xq_sb[:, (ck_hi * EC + e) * 512 + w2:(ck_hi * EC + e + 1) * 512],
                            start=(e == 0), stop=(e == EC - 1),
                            skip_group_check=True)
                if w1 < 512:
                    sl = slice(ck_lo * 512 + w1, (ck_lo + 1) * 512)
                    nc.vector.tensor_copy(q_sb[0:64, sl], pv[0:64, w1:512])
                    nc.scalar.copy(q_sb[64:128, sl], pv[0:64, w1:512])
                if w2 < 512:
                    sl = slice(ck_hi * 512 + w2, (ck_hi + 1) * 512)
                    nc.vector.tensor_copy(q_sb[0:64, sl], pv[64:128, w2:512])
                    nc.scalar.copy(q_sb[64:128, sl], pv[64:128, w2:512])

            def attention(ck, mask_offs):
                tiles = [(jc, mm_off[ck][jc], mk_end[ck][jc])
                         for jc in range(jmax[ck]) if mm_off[ck][jc] < 512]
                if not tiles:
                    return
                po_t = po_pool.tile([H + 1, 512], F32, tag="po")
                first = True
                i = 0
                while i < len(tiles):
                    # two j-tiles share one psum + one exp, but only when the
                    # second tile is wide enough that one big exp (which also
                    # covers the [512:512+qoB] garbage span) beats two exps
                    pair = tiles[i:i + 2]
                    if len(pair) == 2 and pair[1][1] >= 352:
                        pair = tiles[i:i + 1]
                    ps = mms_pool.tile([128, 1024], F32, tag="mms")
                    p_t = p_pool.tile([128, 1024], BF16, tag="p")
                    for h, (jc, qo, me) in enumerate(pair):
                        hf = 64 * (jc % 2)  # row-tiled: alternate array halves
                        nc.tensor.matmul(
                            ps[:, h * 512 + qo:(h + 1) * 512],
                            k_sb[hf:hf + 64, jc * 128:(jc + 1) * 128],
                            q_sb[hf:hf + 64, ck * 512 + qo:(ck + 1) * 512],
                            start=True, stop=True, skip_group_check=True)
                    lo = pair[0][1]
                    hi = (len(pair) - 1) * 512 + 512
                    nc.scalar.activation(
                        p_t[:, lo:hi], ps[:, lo:hi],
                        mybir.ActivationFunctionType.Exp, scale=SCALE)
                    for h, (jc, qo, me) in enumerate(pair):
                        if me > qo:  # boundary mask multiply (host-built content)
                            mo = mask_offs[(ck, jc)]
                            nc.vector.tensor_mul(
                                p_t[:, h * 512 + qo:h * 512 + me],
                                p_t[:, h * 512 + qo:h * 512 + me],
                                msk_sb[:, mo:mo + (me - qo)])
                        nc.tensor.matmul(
                            po_t[:, qo:512],
                            vaug_sb[:, jc * (H + 1):(jc + 1) * (H + 1)],
                            p_t[:, h * 512 + qo:(h + 1) * 512],
                            start=first, stop=(i + h == len(tiles) - 1))
                        first = False
                    i += len(pair)
                # ship unnormalized outT + sums row; the host divides while
                # unsharding (removes the recip chain from the critical tail)
                w0 = mm_off[ck][0]
                o_t = o_pool.tile([H + 1, 512], F32, tag="o")
                nc.scalar.copy(o_t[:, w0:512], po_t[:, w0:512])
                nc.sync.dma_start(
                    out_ext.ap()[:, ck * 512 + w0:(ck + 1) * 512], o_t[:, w0:512])

            # mask tile packing offsets (shared layout; content is per-core)
            mask_offs = {}
            off = 0
            for ck in range(QKC):
                for jc in range(jmax[ck]):
                    qo, me = mm_off[ck][jc], mk_end[ck][jc]
                    if me > qo and qo < 512:
                        mask_offs[(ck, jc)] = off
                        off += me - qo

            # schedule: kv chunks feed attention chunks as soon as possible
            need_kv = [int(np.ceil(jmax[ck] / 4)) for ck in range(QKC)]  # kv chunks needed
            done_kv = 0
            for ck in range(QKC):
                while done_kv < need_kv[ck]:
                    kv_proj(done_kv)
                    done_kv += 1
                if ck == 0:
                    q_proj_pair(0, 0)
                elif ck == 1:
                    q_proj_pair(1, 2)
                attention(ck, mask_offs)
            while done_kv < KC:
                kv_proj(done_kv)
                done_kv += 1

    nc.compile()
    return nc


def _pack_masks(pos, jmax, mm_off, mk_end):
    """Per-core packed boundary masks: msk[j_local, off+q-qo] = (128jc + j_local <= pos[q])."""
    total = 0
    spans = []
    for ck in range(len(jmax)):
        for jc in range(jmax[ck]):
            qo, me = mm_off[ck][jc], mk_end[ck][jc]
            if me > qo and qo < 512:
                spans.append((ck, jc, qo, me, total))
                total += me - qo
    bf = ml_dtypes.bfloat16
    masks = np.zeros((B, 128, max(total, 1)), dtype=np.float32)
    jl = np.arange(128)[:, None]
    for b in range(B):
        for ck, jc, qo, me, off in spans:
            pq = pos[b, ck * 512 + qo: ck * 512 + me][None, :]
            masks[b, :, off:off + (me - qo)] = (128 * jc + jl <= pq)
    return masks.astype(bf), total


def _sbufify(w):  # [E, M] -> [128, EC*M]: w_t[p, e*M+m] = w[e*128+p, m]
    M = w.shape[1]
    return np.ascontiguousarray(
        w.reshape(EC, 128, M).transpose(1, 0, 2).reshape(128, EC * M))


def _retile_cols(xt, ncols):  # [E, ncols] -> [128, (ncols/512)*EC*512] chunk-major
    return np.ascontiguousarray(
        xt.reshape(EC, 128, ncols // 512, 512).transpose(1, 2, 0, 3)
        .reshape(128, (ncols // 512) * EC * 512))


def make_in_maps(x, Wq, Wk, Wv, zero_mask):
    x = np.asarray(x)
    pos, qoff, jmax, mm_off, mk_end = _plan(zero_mask)
    masks, mask_w = _pack_masks(pos, jmax, mm_off, mk_end)
    bf = ml_dtypes.bfloat16
    f8 = ml_dtypes.float8_e4m3fn
    wkv = _sbufify(np.concatenate([np.asarray(Wk), np.asarray(Wv)], 1)).astype(bf)
    wq = _sbufify(np.asarray(Wq)).astype(f8)
    maps = []
    for b in range(B):
        xtb = np.ascontiguousarray(x[b].T.astype(np.float32))
        xqb = np.zeros((E, QK_PAD), dtype=np.float32)
        real = pos[b] >= 0
        xqb[:, real] = xtb[:, pos[b][real]]
        maps.append({
            "xt": _retile_cols(xtb, C).astype(bf),
            "xq": _retile_cols(xqb, QK_PAD).astype(f8),
            "wkv": wkv, "wq": wq, "msk": masks[b],
        })
    return maps, (pos, jmax, mm_off, mk_end, mask_w)


def kernel(x, Wq, Wk, Wv, zero_mask):
    in_maps, (pos, jmax, mm_off, mk_end, mask_w) = make_in_maps(
        x, Wq, Wk, Wv, zero_mask)
    key = (jmax, tuple(map(tuple, mm_off)), tuple(map(tuple, mk_end)), mask_w)
    if _CACHED.get("key") != key:
        _CACHED["nc"] = _build(jmax, mm_off, mk_end, mask_w)
        _CACHED["key"] = key
    res = run_bass_kernel_spmd(_CACHED["nc"], in_maps, core_ids=list(range(B)))
    out = np.zeros((B, C, H), dtype=np.float32)
    for b in range(B):
        r = res.results[b]["out"]  # [H+1, QK_PAD]; row H = softmax denominators
        real = pos[b] >= 0
        out[b][pos[b][real]] = (r[:H, real] / r[H:H + 1, real]).T
    return out
